# revision 23
# baseline (speedup 1.0000x reference)
"""Expert-parallel MoE (top-2 of 8 experts, SwiGLU FFN) on 8 Trainium2 cores.

Strategy
--------
- Router (softmax + top-2 + renormalize + aux loss) is tiny [8192x8] and runs
  on host; it also produces the dispatch plan (which tokens go to which
  expert), which in this full-input/full-output contract IS the all-to-all.
- One expert per core. Each core receives its expert's weights plus the
  tokens routed to it, gathered and stored feature-major (x^T, [D, C]) so the
  PE array never needs an on-device transpose:
    GEMM1: g^T/u^T[F,C] = wg/wu[D,F].T-contract  (lhsT=wg tile, rhs=x^T tile)
    h^T   = silu(g^T) * u^T * gate(col)   (gate fold is legal: GEMM2 linear)
    GEMM2: out^T[D,C]   = wd[F,D].T-contract     (lhsT=wd tile, rhs=h^T tile)
- Matmul operands are bf16 (host-cast); accumulation fp32 in PSUM. Measured
  end-to-end error vs the fp32 reference is ~4e-3 on this data.
- Single resident super-chunk: x^T and h^T stay in SBUF for all C tokens, so
  every weight byte streams from HBM exactly once per call.
"""

import numpy as np
import ml_dtypes

import concourse.bass as bass
import concourse.bacc as bacc
import concourse.tile as tile
from concourse import mybir
from concourse.bass_utils import run_bass_kernel_spmd

B, S, D, F, E = 4, 2048, 2048, 1408, 8
TOP_K = 2
N_TOKENS = B * S
P = 128
NCORES = 8
KT = D // P   # 16 k-tiles over D
FT = F // P   # 11 f-tiles over F
NCH = 64      # capacity granularity
MMF = 1024    # matmul free-dim (tokens per PSUM tile)

f32 = mybir.dt.float32
bf16 = mybir.dt.bfloat16
BF = ml_dtypes.bfloat16

_COMPILED = {}  # capacity C -> bass.Bass program

# Test-harness hooks: set TRACE=True before calling kernel() to profile the
# device execution; the BassKernelResults lands in LAST_RESULT.
TRACE = False
LAST_RESULT = None
LAST_C = None


def _chunks(total: int, step: int):
    """Split `total` into pieces of at most `step`. A tail shorter than 128
    (LDWEIGHTS-bound on the PE) is rebalanced with the preceding piece."""
    sizes = []
    pos = 0
    while pos < total:
        sizes.append(min(step, total - pos))
        pos += sizes[-1]
    if len(sizes) >= 2 and sizes[-1] < 128:
        merged = sizes[-2] + sizes[-1]
        a = ((merged // 2) + 63) // 64 * 64
        sizes[-2:] = [a, merged - a]
    out = []
    pos = 0
    for sz in sizes:
        out.append((pos, sz))
        pos += sz
    return out


def _emit_compute(nc, tc, C, xT, wg, wu, wd, gates, outT):
    """Emit one full forward pass: x^T/gates/weights (DRAM) -> out^T (DRAM)."""
    xT_v = xT.rearrange("(kt p) c -> p kt c", p=P)      # [128, KT, C]
    wg_v = wg.rearrange("(kt p) f -> p kt f", p=P)      # [128, KT, F]
    wu_v = wu.rearrange("(kt p) f -> p kt f", p=P)
    wd_v = wd.rearrange("(ft p) d -> p ft d", p=P)      # [128, FT, D]
    outT_v = outT.rearrange("(dt p) c -> p dt c", p=P)  # [128, KT, C]

    with (
        tc.tile_pool(name="xsc", bufs=1) as xpool,
        tc.tile_pool(name="hsc", bufs=1) as hpool,
        tc.tile_pool(name="wgf", bufs=3) as wgpool,
        tc.tile_pool(name="wuf", bufs=3) as wupool,
        tc.tile_pool(name="wdd", bufs=3) as wdpool,
        tc.tile_pool(name="gat", bufs=1) as gpool,
        tc.tile_pool(name="out", bufs=4) as opool,
        tc.tile_pool(name="ps1", bufs=2, space="PSUM") as ps1,
        tc.tile_pool(name="ps2", bufs=2, space="PSUM") as ps2,
    ):
        x_sc = xpool.tile([P, KT, C], bf16)
        for t0, tsz in _chunks(C, MMF):
            nc.sync.dma_start(
                out=x_sc[:, :, t0:t0 + tsz], in_=xT_v[:, :, t0:t0 + tsz]
            )
        g_sc = gpool.tile([P, C], f32)
        nc.sync.dma_start(out=g_sc, in_=gates[:, :].partition_broadcast(P))
        h_sc = hpool.tile([P, FT, C], bf16)

        # ---- GEMM1 + SwiGLU + gate: h^T = silu(x@wg)^T * (x@wu)^T * gate
        for f in range(FT):
            wg_f = wgpool.tile([P, KT, P], bf16)
            nc.sync.dma_start(out=wg_f, in_=wg_v[:, :, f * P:(f + 1) * P])
            wu_f = wupool.tile([P, KT, P], bf16)
            nc.sync.dma_start(out=wu_f, in_=wu_v[:, :, f * P:(f + 1) * P])
            for t0, tsz in _chunks(C, MMF):
                ts = slice(t0, t0 + tsz)
                pg_full = ps1.tile([P, MMF], f32, tag="ps1")
                pg = pg_full[:, :tsz]
                for k in range(KT):
                    nc.tensor.matmul(
                        pg, wg_f[:, k, :], x_sc[:, k, ts],
                        start=(k == 0), stop=(k == KT - 1),
                    )
                ht = h_sc[:, f, ts]
                nc.scalar.activation(
                    ht, pg, mybir.ActivationFunctionType.Silu
                )
                pu_full = ps1.tile([P, MMF], f32, tag="ps1")
                pu = pu_full[:, :tsz]
                for k in range(KT):
                    nc.tensor.matmul(
                        pu, wu_f[:, k, :], x_sc[:, k, ts],
                        start=(k == 0), stop=(k == KT - 1),
                    )
                nc.vector.tensor_mul(ht, ht, pu)
                nc.vector.tensor_mul(ht, ht, g_sc[:, ts])

        # ---- GEMM2: out^T[dtile, tokens] = sum_f wd^T h^T
        for dt_i in range(KT):
            wd_d = wdpool.tile([P, FT, P], bf16)
            nc.sync.dma_start(out=wd_d, in_=wd_v[:, :, dt_i * P:(dt_i + 1) * P])
            for t0, tsz in _chunks(C, MMF):
                pd_full = ps2.tile([P, MMF], f32, tag="ps2")
                pd = pd_full[:, :tsz]
                for f in range(FT):
                    nc.tensor.matmul(
                        pd, wd_d[:, f, :], h_sc[:, f, t0:t0 + tsz],
                        start=(f == 0), stop=(f == FT - 1),
                    )
                ot_full = opool.tile([P, MMF], f32, tag="ot")
                ot = ot_full[:, :tsz]
                nc.vector.tensor_copy(ot, pd)
                nc.sync.dma_start(
                    out=outT_v[:, dt_i, t0:t0 + tsz], in_=ot,
                )


def _build_program(C: int, loop_n: int = 1) -> bass.Bass:
    """Per-core program: x^T[D,C] x one expert's weights -> out^T[D,C].

    loop_n > 1 wraps the body in a hardware loop (timing runs only)."""
    assert C % NCH == 0

    nc = bacc.Bacc("TRN2", target_bir_lowering=False, debug=False)
    xT = nc.declare_dram_parameter("xT", [D, C], bf16, isOutput=False)
    wg = nc.declare_dram_parameter("wg", [D, F], bf16, isOutput=False)
    wu = nc.declare_dram_parameter("wu", [D, F], bf16, isOutput=False)
    wd = nc.declare_dram_parameter("wd", [F, D], bf16, isOutput=False)
    gates = nc.declare_dram_parameter("gates", [1, C], f32, isOutput=False)
    outT = nc.declare_dram_parameter("outT", [D, C], f32, isOutput=True)

    with tile.TileContext(nc) as tc:
        if loop_n > 1:
            with tc.For_i(0, loop_n, 1):
                _emit_compute(nc, tc, C, xT, wg, wu, wd, gates, outT)
        else:
            _emit_compute(nc, tc, C, xT, wg, wu, wd, gates, outT)
    nc.finalize()
    return nc


def _build_bench_program(C: int) -> bass.Bass:
    """Timing variant: identical compute on internal (unshipped) DRAM
    tensors, iterated n times in a hardware loop where n arrives as DATA
    (one NEFF for every n). Wall-clock slope over n isolates pure device
    time per iteration from every per-call constant."""
    assert C % NCH == 0
    nc = bacc.Bacc("TRN2", target_bir_lowering=False, debug=False)
    i32 = mybir.dt.int32
    nreps = nc.declare_dram_parameter("nreps", [1, 1], i32, isOutput=False)
    tok_out = nc.declare_dram_parameter("tok_out", [1, 1], f32, isOutput=True)
    xT = nc.dram_tensor("xTi", [D, C], bf16)
    wg = nc.dram_tensor("wgi", [D, F], bf16)
    wu = nc.dram_tensor("wui", [D, F], bf16)
    wd = nc.dram_tensor("wdi", [F, D], bf16)
    gates = nc.dram_tensor("gatesi", [1, C], f32)
    outT = nc.dram_tensor("outTi", [D, C], f32)

    with tile.TileContext(nc) as tc:
        with tc.tile_pool(name="tok", bufs=1) as tpool:
            nt = tpool.tile([1, 1], i32)
            nc.sync.dma_start(out=nt, in_=nreps[:, :])
            nv = nc.values_load(nt[0:1, 0:1], min_val=0, max_val=1 << 20,
                                skip_runtime_bounds_check=True)
            with tc.For_i(0, nv, 1, staggered_reset=True):
                _emit_compute(nc, tc, C, xT, wg, wu, wd, gates, outT)
            tout = tpool.tile([1, 1], f32)
            nc.vector.memset(tout, 1.0)
            nc.sync.dma_start(out=tok_out[:, :], in_=tout)
    nc.finalize()
    return nc


def _router_host(x: np.ndarray, router_w: np.ndarray):
    """Replicate the jax reference router in numpy fp32."""
    logits = x @ router_w                                   # [N, E] fp32
    m = logits.max(axis=-1, keepdims=True)
    e = np.exp(logits - m)
    probs = e / e.sum(axis=-1, keepdims=True)
    # top-2, ties -> lower index first (matches jax.lax.top_k)
    top_i = np.argsort(-probs, axis=-1, kind="stable")[:, :TOP_K]
    top_w = np.take_along_axis(probs, top_i, axis=-1)
    top_w = top_w / top_w.sum(axis=-1, keepdims=True)

    counts = np.zeros(E, dtype=np.int64)
    for k in range(TOP_K):
        counts += np.bincount(top_i[:, k], minlength=E)
    f = counts.astype(np.float32) / np.float32(x.shape[0])
    Pm = probs.mean(axis=0, dtype=np.float32)
    aux_loss = np.float32(E / TOP_K) * np.float32(np.sum(f * Pm, dtype=np.float32))
    return top_i, top_w.astype(np.float32), aux_loss


def kernel(hidden_states, router_w, w_gate, w_up, w_down):
    x = np.ascontiguousarray(hidden_states.reshape(-1, D)).astype(np.float32)
    N = x.shape[0]

    top_i, top_w, aux_loss = _router_host(x, router_w.astype(np.float32))

    # Dispatch: token lists per expert (a token hits an expert at most once).
    idx_per_e = []
    gate_per_e = []
    max_count = 0
    for e in range(E):
        hits = np.nonzero(top_i == e)
        idx = hits[0]
        gv = top_w[hits[0], hits[1]]
        idx_per_e.append(idx)
        gate_per_e.append(gv)
        max_count = max(max_count, len(idx))
    C = max(NCH, ((max_count + NCH - 1) // NCH) * NCH)

    global LAST_C
    LAST_C = C
    if C not in _COMPILED:
        _COMPILED[C] = _build_program(C)
    nc = _COMPILED[C]

    in_maps = []
    for e in range(E):
        idx = idx_per_e[e]
        xT_e = np.zeros((D, C), dtype=BF)
        xT_e[:, : len(idx)] = x[idx].astype(BF).T
        g_e = np.zeros((1, C), dtype=np.float32)
        g_e[0, : len(idx)] = gate_per_e[e]
        in_maps.append({
            "xT": xT_e,
            "wg": np.ascontiguousarray(w_gate[e]).astype(BF),
            "wu": np.ascontiguousarray(w_up[e]).astype(BF),
            "wd": np.ascontiguousarray(w_down[e]).astype(BF),
            "gates": g_e,
        })

    res = run_bass_kernel_spmd(nc, in_maps, list(range(NCORES)), trace=TRACE)
    global LAST_RESULT
    LAST_RESULT = res

    out = np.zeros((N, D), dtype=np.float32)
    for e in range(E):
        idx = idx_per_e[e]
        out[idx] += res.results[e]["outT"][:, : len(idx)].T
    return out.reshape(B, S, D), aux_loss


# revision 25
# speedup vs baseline: 1.1269x; 1.1269x over previous
"""Expert-parallel MoE (top-2 of 8 experts, SwiGLU FFN) on 8 Trainium2 cores.

Strategy
--------
- Router (softmax + top-2 + renormalize + aux loss) is tiny [8192x8] and runs
  on host; it also produces the dispatch plan (which tokens go to which
  expert), which in this full-input/full-output contract IS the all-to-all.
- One expert per core. Each core receives its expert's weights plus the
  tokens routed to it, gathered and stored feature-major (x^T, [D, C]) so the
  PE array never needs an on-device transpose:
    GEMM1: g^T/u^T[F,C] = wg/wu[D,F].T-contract  (lhsT=wg tile, rhs=x^T tile)
    h^T   = silu(g^T) * u^T * gate(col)   (gate fold is legal: GEMM2 linear)
    GEMM2: out^T[D,C]   = wd[F,D].T-contract     (lhsT=wd tile, rhs=h^T tile)
- Matmul operands are bf16 (host-cast); accumulation fp32 in PSUM. Measured
  end-to-end error vs the fp32 reference is ~4e-3 on this data.
- Single resident super-chunk: x^T and h^T stay in SBUF for all C tokens, so
  every weight byte streams from HBM exactly once per call.
"""

import numpy as np
import ml_dtypes

import concourse.bass as bass
import concourse.bacc as bacc
import concourse.tile as tile
from concourse import mybir
from concourse.bass_utils import run_bass_kernel_spmd

B, S, D, F, E = 4, 2048, 2048, 1408, 8
TOP_K = 2
N_TOKENS = B * S
P = 128
NCORES = 8
KT = D // P   # 16 k-tiles over D
FT = F // P   # 11 f-tiles over F
NCH = 64      # capacity granularity
MMF = 1024    # matmul free-dim (tokens per PSUM tile)

f32 = mybir.dt.float32
bf16 = mybir.dt.bfloat16
BF = ml_dtypes.bfloat16

_COMPILED = {}  # capacity C -> bass.Bass program

# Test-harness hooks: set TRACE=True before calling kernel() to profile the
# device execution; the BassKernelResults lands in LAST_RESULT.
TRACE = False
LAST_RESULT = None
LAST_C = None


def _chunks(total: int, step: int):
    """Split `total` into pieces of at most `step`. A tail shorter than 128
    (LDWEIGHTS-bound on the PE) is rebalanced with the preceding piece."""
    sizes = []
    pos = 0
    while pos < total:
        sizes.append(min(step, total - pos))
        pos += sizes[-1]
    if len(sizes) >= 2 and sizes[-1] < 128:
        merged = sizes[-2] + sizes[-1]
        a = ((merged // 2) + 63) // 64 * 64
        sizes[-2:] = [a, merged - a]
    out = []
    pos = 0
    for sz in sizes:
        out.append((pos, sz))
        pos += sz
    return out


def _emit_compute(nc, tc, C, xT, wg, wu, wd, gates, outT):
    """Emit one full forward pass: x^T/gates/weights (DRAM) -> out^T (DRAM)."""
    xT_v = xT.rearrange("(kt p) c -> p kt c", p=P)      # [128, KT, C]
    wg_v = wg.rearrange("(kt p) f -> p kt f", p=P)      # [128, KT, F]
    wu_v = wu.rearrange("(kt p) f -> p kt f", p=P)
    wd_v = wd.rearrange("(ft p) d -> p ft d", p=P)      # [128, FT, D]
    outT_v = outT.rearrange("(dt p) c -> p dt c", p=P)  # [128, KT, C]

    with (
        tc.tile_pool(name="xsc", bufs=1) as xpool,
        tc.tile_pool(name="hsc", bufs=1) as hpool,
        tc.tile_pool(name="wgf", bufs=3) as wgpool,
        tc.tile_pool(name="wuf", bufs=3) as wupool,
        tc.tile_pool(name="wdd", bufs=3) as wdpool,
        tc.tile_pool(name="gat", bufs=1) as gpool,
        tc.tile_pool(name="out", bufs=4) as opool,
        tc.tile_pool(name="ps1", bufs=2, space="PSUM") as ps1,
        tc.tile_pool(name="ps2", bufs=2, space="PSUM") as ps2,
    ):
        x_sc = xpool.tile([P, KT, C], bf16)
        for t0, tsz in _chunks(C, MMF):
            nc.sync.dma_start(
                out=x_sc[:, :, t0:t0 + tsz], in_=xT_v[:, :, t0:t0 + tsz]
            )
        g_sc = gpool.tile([P, C], f32)
        nc.sync.dma_start(out=g_sc, in_=gates[:, :].partition_broadcast(P))
        h_sc = hpool.tile([P, FT, C], bf16)

        # ---- GEMM1 + SwiGLU + gate: h^T = silu(x@wg)^T * (x@wu)^T * gate
        for f in range(FT):
            wg_f = wgpool.tile([P, KT, P], bf16)
            nc.sync.dma_start(out=wg_f, in_=wg_v[:, :, f * P:(f + 1) * P])
            wu_f = wupool.tile([P, KT, P], bf16)
            nc.sync.dma_start(out=wu_f, in_=wu_v[:, :, f * P:(f + 1) * P])
            for t0, tsz in _chunks(C, MMF):
                ts = slice(t0, t0 + tsz)
                pg_full = ps1.tile([P, MMF], f32, tag="ps1")
                pg = pg_full[:, :tsz]
                for k in range(KT):
                    nc.tensor.matmul(
                        pg, wg_f[:, k, :], x_sc[:, k, ts],
                        start=(k == 0), stop=(k == KT - 1),
                    )
                ht = h_sc[:, f, ts]
                nc.scalar.activation(
                    ht, pg, mybir.ActivationFunctionType.Silu
                )
                pu_full = ps1.tile([P, MMF], f32, tag="ps1")
                pu = pu_full[:, :tsz]
                for k in range(KT):
                    nc.tensor.matmul(
                        pu, wu_f[:, k, :], x_sc[:, k, ts],
                        start=(k == 0), stop=(k == KT - 1),
                    )
                nc.vector.tensor_mul(ht, ht, pu)

        # ---- GEMM2: out^T[dtile, tokens] = sum_f wd^T h^T
        for dt_i in range(KT):
            wd_d = wdpool.tile([P, FT, P], bf16)
            nc.sync.dma_start(out=wd_d, in_=wd_v[:, :, dt_i * P:(dt_i + 1) * P])
            for t0, tsz in _chunks(C, MMF):
                pd_full = ps2.tile([P, MMF], f32, tag="ps2")
                pd = pd_full[:, :tsz]
                for f in range(FT):
                    nc.tensor.matmul(
                        pd, wd_d[:, f, :], h_sc[:, f, t0:t0 + tsz],
                        start=(f == 0), stop=(f == FT - 1),
                    )
                ot_full = opool.tile([P, MMF], f32, tag="ot")
                ot = ot_full[:, :tsz]
                nc.vector.tensor_mul(ot, pd, g_sc[:, t0:t0 + tsz])
                nc.sync.dma_start(
                    out=outT_v[:, dt_i, t0:t0 + tsz], in_=ot,
                )


def _build_program(C: int, loop_n: int = 1) -> bass.Bass:
    """Per-core program: x^T[D,C] x one expert's weights -> out^T[D,C].

    loop_n > 1 wraps the body in a hardware loop (timing runs only)."""
    assert C % NCH == 0

    nc = bacc.Bacc("TRN2", target_bir_lowering=False, debug=False)
    xT = nc.declare_dram_parameter("xT", [D, C], bf16, isOutput=False)
    wg = nc.declare_dram_parameter("wg", [D, F], bf16, isOutput=False)
    wu = nc.declare_dram_parameter("wu", [D, F], bf16, isOutput=False)
    wd = nc.declare_dram_parameter("wd", [F, D], bf16, isOutput=False)
    gates = nc.declare_dram_parameter("gates", [1, C], f32, isOutput=False)
    outT = nc.declare_dram_parameter("outT", [D, C], f32, isOutput=True)

    with tile.TileContext(nc) as tc:
        if loop_n > 1:
            with tc.For_i(0, loop_n, 1):
                _emit_compute(nc, tc, C, xT, wg, wu, wd, gates, outT)
        else:
            _emit_compute(nc, tc, C, xT, wg, wu, wd, gates, outT)
    nc.finalize()
    return nc


def _build_bench_program(C: int) -> bass.Bass:
    """Timing variant: identical compute on internal (unshipped) DRAM
    tensors, iterated n times in a hardware loop where n arrives as DATA
    (one NEFF for every n). Wall-clock slope over n isolates pure device
    time per iteration from every per-call constant."""
    assert C % NCH == 0
    nc = bacc.Bacc("TRN2", target_bir_lowering=False, debug=False)
    i32 = mybir.dt.int32
    nreps = nc.declare_dram_parameter("nreps", [1, 1], i32, isOutput=False)
    tok_out = nc.declare_dram_parameter("tok_out", [1, 1], f32, isOutput=True)
    xT = nc.dram_tensor("xTi", [D, C], bf16)
    wg = nc.dram_tensor("wgi", [D, F], bf16)
    wu = nc.dram_tensor("wui", [D, F], bf16)
    wd = nc.dram_tensor("wdi", [F, D], bf16)
    gates = nc.dram_tensor("gatesi", [1, C], f32)
    outT = nc.dram_tensor("outTi", [D, C], f32)

    with tile.TileContext(nc) as tc:
        with tc.tile_pool(name="tok", bufs=1) as tpool:
            nt = tpool.tile([1, 1], i32)
            nc.sync.dma_start(out=nt, in_=nreps[:, :])
            nv = nc.values_load(nt[0:1, 0:1], min_val=0, max_val=1 << 20,
                                skip_runtime_bounds_check=True)
            with tc.For_i(0, nv, 1):
                _emit_compute(nc, tc, C, xT, wg, wu, wd, gates, outT)
            tout = tpool.tile([1, 1], f32)
            nc.vector.memset(tout, 1.0)
            nc.sync.dma_start(out=tok_out[:, :], in_=tout)
    nc.finalize()
    return nc


def _router_host(x: np.ndarray, router_w: np.ndarray):
    """Replicate the jax reference router in numpy fp32."""
    logits = x @ router_w                                   # [N, E] fp32
    m = logits.max(axis=-1, keepdims=True)
    e = np.exp(logits - m)
    probs = e / e.sum(axis=-1, keepdims=True)
    # top-2, ties -> lower index first (matches jax.lax.top_k)
    top_i = np.argsort(-probs, axis=-1, kind="stable")[:, :TOP_K]
    top_w = np.take_along_axis(probs, top_i, axis=-1)
    top_w = top_w / top_w.sum(axis=-1, keepdims=True)

    counts = np.zeros(E, dtype=np.int64)
    for k in range(TOP_K):
        counts += np.bincount(top_i[:, k], minlength=E)
    f = counts.astype(np.float32) / np.float32(x.shape[0])
    Pm = probs.mean(axis=0, dtype=np.float32)
    aux_loss = np.float32(E / TOP_K) * np.float32(np.sum(f * Pm, dtype=np.float32))
    return top_i, top_w.astype(np.float32), aux_loss


def kernel(hidden_states, router_w, w_gate, w_up, w_down):
    x = np.ascontiguousarray(hidden_states.reshape(-1, D)).astype(np.float32)
    N = x.shape[0]

    top_i, top_w, aux_loss = _router_host(x, router_w.astype(np.float32))

    # Dispatch: token lists per expert (a token hits an expert at most once).
    idx_per_e = []
    gate_per_e = []
    max_count = 0
    for e in range(E):
        hits = np.nonzero(top_i == e)
        idx = hits[0]
        gv = top_w[hits[0], hits[1]]
        idx_per_e.append(idx)
        gate_per_e.append(gv)
        max_count = max(max_count, len(idx))
    C = max(NCH, ((max_count + NCH - 1) // NCH) * NCH)

    global LAST_C
    LAST_C = C
    if C not in _COMPILED:
        _COMPILED[C] = _build_program(C)
    nc = _COMPILED[C]

    in_maps = []
    for e in range(E):
        idx = idx_per_e[e]
        xT_e = np.zeros((D, C), dtype=BF)
        xT_e[:, : len(idx)] = x[idx].astype(BF).T
        g_e = np.zeros((1, C), dtype=np.float32)
        g_e[0, : len(idx)] = gate_per_e[e]
        in_maps.append({
            "xT": xT_e,
            "wg": np.ascontiguousarray(w_gate[e]).astype(BF),
            "wu": np.ascontiguousarray(w_up[e]).astype(BF),
            "wd": np.ascontiguousarray(w_down[e]).astype(BF),
            "gates": g_e,
        })

    res = run_bass_kernel_spmd(nc, in_maps, list(range(NCORES)), trace=TRACE)
    global LAST_RESULT
    LAST_RESULT = res

    out = np.zeros((N, D), dtype=np.float32)
    for e in range(E):
        idx = idx_per_e[e]
        out[idx] += res.results[e]["outT"][:, : len(idx)].T
    return out.reshape(B, S, D), aux_loss
